# revision 6
# baseline (speedup 1.0000x reference)
"""Trainium2 Bass kernel for nn_Attention_11055245820093.

Swin-style attention block: qkv proj -> per-head scaled dot-product attention
with 2D relative position bias (CLS zero-padded), per-head softplus temperature,
patch-diagonal mask -> proj.

Strategy: data-parallel over batch B=64 across 8 NeuronCores (8 batches/core).
All compute per core runs in a "transposed" layout (channels on partitions,
tokens on the free dim) so no on-device transposes are needed.

Numerics (validated at rel_err ~5e-3): projections in fp8e4m3 DoubleRow with
full error compensation (hi*hi pairs + interleaved cross terms, 9 DR
instructions per 768-contraction tile = 1.33x bf16); attention (QK^T, exp,
bias, AV, softmax divide) in bf16 -- every UNcompensated e4m3 activation
quantization alone costs ~2.5e-2 max-rel (over the 2e-2 gate), so fp8
attention is not affordable.

v3 performance structure (PE busy ~114us is the cost-model floor for this
algorithm; the work is eliminating PE idle):
  - ONE flat software-pipelined loop over (head-pair, batch): iteration g
    issues S[g] (QK^T), then PE "filler" units, then AV[g-1] -- AV runs one
    iteration behind so its exp/bias-mul input chain (ACT+Pool) has a full
    iteration of slack and PE never stalls on it.
  - PE filler units drawn from a static earliest-deadline schedule: V-proj
    units B[b] (due before AV uses V[b]), QK-proj subtiles (due before the
    head-pair's S reads that token window), and output-proj groups D(nt, mt)
    (ready once the last head-pair's norms for window nt completed).
  - bias multiply (e = exp(S) * ebias) runs on GPSIMD/Pool (proxy library
    tensor_tensor, SBUF-only) -- off the DVE critical path.
  - QK-proj PSUM uses token-order window offsets so each eviction is ONE
    contiguous [128, 394] op (the baseline's window-permuted layout needed
    two strided ops).
  - evictions alternate ACT/DVE to balance; outT DMAs round-robin across
    engine queues (they serialized on SP in the baseline, 3.6us tail).
"""

import os
import sys

sys.path.insert(0, "/opt/trn_rl_repo")
os.environ.setdefault("MYCRO_LOCAL_CACHE", "1")

import numpy as np
import ml_dtypes

BF16 = ml_dtypes.bfloat16
F8 = ml_dtypes.float8_e4m3fn

# Problem constants (hardcoded per contract)
B, N, C, H, D = 64, 197, 768, 12, 64
NCORES = 8
BPC = B // NCORES          # 8 batches per core
T = BPC * N                # 1576 tokens per core
KT = C // 128              # 6 contraction tiles of 128
NT = 4                     # token n-tiles
TN = T // NT               # 394 tokens per n-tile
SCALE = D ** -0.5
JROWS = (128, N - 128)     # 128, 69
N2 = 2 * N
SVF = 64.0                 # wv host-scale; ones column matches so the
                           # softmax divide cancels it exactly
SQ, SK = 256.0, 64.0

_CACHE = {}

TRACE = False
LAST_RESULTS = None


def _build(finalize=True):
    import concourse.bass as bass
    import concourse.tile as tile
    from concourse import bacc, library_config, mybir

    dt = mybir.dt
    f32, bf16, f8 = dt.float32, dt.bfloat16, dt.float8e4
    AF = mybir.ActivationFunctionType
    OP = mybir.AluOpType
    DR = mybir.MatmulPerfMode.DoubleRow

    nc = bacc.Bacc("TRN2", target_bir_lowering=False, debug=False)

    x_hi = nc.dram_tensor(
        "x_hi", [NT, 128, 4, KT // 2, 2, 128], f8, kind="ExternalInput"
    ).ap()
    x_x = nc.dram_tensor(
        "x_x", [NT, 128, 4, KT, 2, 128], f8, kind="ExternalInput"
    ).ap()
    wv_x = nc.dram_tensor(
        "wv_x", [2, 128, KT, 2, C // 2], f8, kind="ExternalInput"
    ).ap()
    wqk_hi = nc.dram_tensor(
        "wqk_hi", [128, 2 * KT, KT // 2, 2, 128], f8, kind="ExternalInput"
    ).ap()
    wqk_x = nc.dram_tensor(
        "wqk_x", [128, 2 * KT, KT, 2, 128], f8, kind="ExternalInput"
    ).ap()
    wpj = nc.dram_tensor("wpj", [KT, 128, C], bf16, kind="ExternalInput").ap()
    bT = nc.dram_tensor("bT", [KT, N, 2 * N], bf16, kind="ExternalInput").ap()
    bqk = nc.dram_tensor("bqk", [128, 2 * KT], f32, kind="ExternalInput").ap()
    outT = nc.dram_tensor("outT", [KT, 128, T], f32, kind="ExternalOutput").ap()

    # token-order psum window offsets: w index = xh memory order
    # [w0(tok 0:128), w2(tok 197:325), w1(tok 128:197), w3(tok 325:394)]
    WMO = ((0, 128), (197, 128), (128, 69), (325, 69))

    with tile.TileContext(nc) as tc:
        from contextlib import ExitStack

        with ExitStack() as ctx:
            nc.gpsimd.load_library(library_config.proxy)
            cp = ctx.enter_context(tc.tile_pool(name="consts", bufs=1))
            psA = ctx.enter_context(tc.tile_pool(name="psA", bufs=2, space="PSUM"))
            psC = ctx.enter_context(tc.tile_pool(name="psC", bufs=3, space="PSUM"))
            wp = ctx.enter_context(tc.tile_pool(name="work", bufs=2))

            # ---- persistent SBUF tiles; DMAs in consumption order ----
            xh_sb = cp.tile([128, NT, 4, KT // 2, 2, 128], f8, name="xh", tag="xh")
            xx_sb = cp.tile([128, NT, 4, KT, 2, 128], f8, name="xx", tag="xx")
            wvx_sb = cp.tile([128, 2, KT, 2, C // 2], f8, name="wvx", tag="wvx")
            wqkh_sb = cp.tile(
                [128, 2 * KT, KT // 2, 2, 128], f8, name="wqkh", tag="wqkh"
            )
            wqkx_sb = cp.tile(
                [128, 2 * KT, KT, 2, 128], f8, name="wqkx", tag="wqkx"
            )
            bqk_sb = cp.tile([128, 2 * KT], f32, name="bqk", tag="bqk")
            # startup-critical DMAs on separate queues: B units need
            # xh0/xx0/wvx; the prologue A units need wqkh/wqkx/bqk
            nc.gpsimd.dma_start(out=wvx_sb[:, 0], in_=wv_x[0])
            nc.gpsimd.dma_start(out=wvx_sb[:, 1], in_=wv_x[1])
            nc.scalar.dma_start(out=xh_sb[:, 0], in_=x_hi[0])
            nc.scalar.dma_start(out=xx_sb[:, 0], in_=x_x[0])
            nc.sync.dma_start(out=wqkh_sb[:], in_=wqk_hi[:])
            nc.sync.dma_start(out=wqkx_sb[:, 0:KT], in_=wqk_x[:, 0:KT])
            nc.gpsimd.dma_start(out=wqkx_sb[:, KT:], in_=wqk_x[:, KT:])
            nc.scalar.dma_start(out=bqk_sb[:], in_=bqk[:])
            for nt in range(1, NT):
                nc.sync.dma_start(out=xh_sb[:, nt], in_=x_hi[nt])
                nc.sync.dma_start(out=xx_sb[:, nt], in_=x_x[nt])
            bias_sb = {}
            for hp in range(KT):
                for jt, rows in enumerate(JROWS):
                    bias_sb[(hp, jt)] = cp.tile(
                        [rows, N2], bf16, name=f"bias{hp}_{jt}", tag=f"bias{hp}_{jt}"
                    )
            wpj_sb = [
                cp.tile([128, C], bf16, name=f"wpj{k}", tag=f"wpj{k}")
                for k in range(KT)
            ]

            # qk tiles: Q (mt 0..5), K (mt 6..11)
            qk_sb = [
                cp.tile([128, T], bf16, name=f"qk{m}", tag=f"qk{m}")
                for m in range(2 * KT)
            ]
            # V per (batch, jt): (rows, 12 heads, 65) -- 64 V cols + ones col
            v_sb = {}
            for b in range(BPC):
                for jt, rows in enumerate(JROWS):
                    t_ = cp.tile(
                        [rows, H, D + 1], bf16, name=f"v{b}_{jt}", tag=f"v{b}_{jt}"
                    )
                    nc.vector.memset(t_[:, :, D : D + 1], SVF)
                    v_sb[(b, jt)] = t_
            attn_sb = [
                cp.tile([128, T], bf16, name=f"at{m}", tag=f"at{m}") for m in range(KT)
            ]

            evict_flip = [0]

            def evict_engine():
                evict_flip[0] += 1
                return nc.vector if evict_flip[0] % 3 == 0 else nc.scalar

            # ---- filler unit emitters (pure PE work + one eviction) ----
            def unit_b(b, jt, n2):
                """V-proj quarter: one psum group -> v_sb[(b, jt)] slice."""
                rows = JROWS[jt]
                ntb = b // 2
                wpos = 2 * jt + (b % 2)
                psv = psA.tile([128, 512], f32, tag="psA")
                for p in range(KT // 2):
                    nc.tensor.matmul(
                        psv[0:128, 0 : C // 2],
                        xh_sb[:, ntb, wpos, p, :, :],
                        wvx_sb[:, n2, 2 * p : 2 * p + 2, 1, :],
                        start=(p == 0),
                        stop=False,
                        perf_mode=DR,
                    )
                for k in range(KT):
                    nc.tensor.matmul(
                        psv[0:128, 0 : C // 2],
                        xx_sb[:, ntb, wpos, k, :, :],
                        wvx_sb[:, n2, k, :, :],
                        start=False,
                        stop=(k == KT - 1),
                        perf_mode=DR,
                    )
                eng = evict_engine()
                dst = v_sb[(b, jt)][0:rows, n2 * KT : (n2 + 1) * KT, 0:D]
                src = psv[0:rows, 0 : C // 2].rearrange("p (h d) -> p h d", h=KT)
                if eng is nc.vector:
                    nc.vector.tensor_copy(dst, src)
                else:
                    nc.scalar.activation(dst, src, AF.Copy)

            def unit_a(mt, nt):
                """QK-proj subtile: one token window of Q or K tile mt."""
                inv_s = (1.0 / SQ) if mt < KT else (1.0 / SK)
                ps = psA.tile([128, 512], f32, tag="psA")
                first = True
                for p in range(KT // 2):
                    for w, (o, gl) in enumerate(WMO):
                        nc.tensor.matmul(
                            ps[:, o : o + gl],
                            wqkh_sb[:, mt, p, :, :],
                            xh_sb[:, nt, w, p, :, 0:gl],
                            start=first,
                            stop=False,
                            perf_mode=DR,
                        )
                        first = False
                for k in range(KT):
                    for w, (o, gl) in enumerate(WMO):
                        nc.tensor.matmul(
                            ps[:, o : o + gl],
                            wqkx_sb[:, mt, k, :, :],
                            xx_sb[:, nt, w, k, :, 0:gl],
                            start=False,
                            stop=(k == KT - 1 and w == 3),
                            perf_mode=DR,
                        )
                # token-order psum -> single contiguous eviction
                dst = qk_sb[mt][:, nt * TN : (nt + 1) * TN]
                if evict_engine() is nc.vector:
                    nc.vector.tensor_scalar(
                        dst, ps[:, 0:TN], inv_s, bqk_sb[:, mt : mt + 1],
                        OP.mult, OP.add,
                    )
                else:
                    nc.scalar.activation(
                        dst, ps[:, 0:TN], AF.Identity,
                        bias=bqk_sb[:, mt : mt + 1], scale=inv_s,
                    )

            dq = [0]

            def unit_d(nt, mt):
                """output-proj group: one (window, out-tile) -> outT DMA."""
                ps = psA.tile([128, 512], f32, tag="psA")
                for k in range(KT):
                    nc.tensor.matmul(
                        ps[:, 0:TN],
                        wpj_sb[k][:, mt * 128 : (mt + 1) * 128],
                        attn_sb[k][:, nt * TN : (nt + 1) * TN],
                        start=(k == 0),
                        stop=(k == KT - 1),
                    )
                ot = wp.tile([128, TN], f32, tag="ot", bufs=3)
                if evict_engine() is nc.vector:
                    nc.vector.tensor_copy(ot[:], ps[:, 0:TN])
                else:
                    nc.scalar.activation(ot[:], ps[:, 0:TN], AF.Copy)
                q = (nc.sync, nc.scalar, nc.gpsimd)[dq[0] % 3]
                dq[0] += 1
                q.dma_start(out=outT[mt, :, nt * TN : (nt + 1) * TN], in_=ot[:])

            # ---- attention pieces ----
            s_tiles = {}

            def emit_s(g):
                """S^T matmuls + exp + Pool bias-mul for iteration g."""
                hp, b = divmod(g, BPC)
                e2 = wp.tile([128, 2, 2, N], bf16, tag="e2", bufs=3)
                for jt, rows in enumerate(JROWS):
                    ps = psC.tile([128, 2, 512], f32, tag="psC")
                    for hh in range(2):
                        base = 64 * hh
                        nc.tensor.matmul(
                            ps[0:rows, hh, 0:N],
                            qk_sb[KT + hp][
                                base : base + 64,
                                b * N + jt * 128 : b * N + jt * 128 + rows,
                            ],
                            qk_sb[hp][base : base + 64, b * N : (b + 1) * N],
                            start=True,
                            stop=True,
                        )
                    eu = wp.tile([128, 2, N], bf16, tag=f"eu{jt}", bufs=3)
                    nc.scalar.activation(
                        eu[0:rows, :, :], ps[0:rows, :, 0:N], AF.Exp
                    )
                    # multiplicative rel-pos bias (exp'd on host) on Pool
                    nc.gpsimd.tensor_mul(
                        e2[0:rows, :, jt, :],
                        eu[0:rows, :, :],
                        bias_sb[(hp, jt)][0:rows, :].rearrange(
                            "p (g n) -> p g n", g=2
                        ),
                    )
                s_tiles[g] = e2

            def emit_av(g):
                """AV + softmax normalize for iteration g (runs at g+1)."""
                hp, b = divmod(g, BPC)
                e2 = s_tiles.pop(g)
                po = psC.tile([128, 2, 512], f32, tag="psC")
                for hh in range(2):
                    h = 2 * hp + hh
                    for jt, rows in enumerate(JROWS):
                        nc.tensor.matmul(
                            po[0 : D + 1, hh, 0:N],
                            v_sb[(b, jt)][0:rows, h, 0 : D + 1],
                            e2[0:rows, hh, jt, :],
                            start=(jt == 0),
                            stop=(jt == 1),
                        )
                r2 = wp.tile([1, 2, N], bf16, tag="r2", bufs=3)
                with nc.allow_low_precision(
                    reason="softmax denom reciprocal in bf16"
                ):
                    nc.vector.reciprocal(r2[:, :, :], po[D : D + 1, :, 0:N])
                rb = wp.tile([128, N2], bf16, tag="rb", bufs=3)
                nc.gpsimd.partition_broadcast(rb[:, :], r2[:, :, :])
                nc.vector.tensor_mul(
                    attn_sb[hp][0:D, b * N : (b + 1) * N],
                    po[0:D, 0, 0:N],
                    rb[0:D, 0:N],
                )
                nc.vector.tensor_mul(
                    attn_sb[hp][D : 2 * D, b * N : (b + 1) * N],
                    po[0:D, 1, 0:N],
                    rb[D : 2 * D, N:N2],
                )

            # ---- static filler schedule ----
            # AB units have HARD deadlines (due = first iteration whose S or
            # AV reads their output; emitting later would cycle the in-order
            # PE queue through an ACT/DVE eviction that sits behind stalled
            # work -> deadlock).  D units instead have a READY iteration
            # (earliest emission keeping attn writes ahead in PE order).
            abunits = []
            for b in range(2, BPC):
                for jt in range(2):
                    for n2 in range(2):
                        # V[b] consumed by AV[b] emitted at iteration b+1
                        abunits.append(
                            (b, lambda b=b, jt=jt, n2=n2: unit_b(b, jt, n2))
                        )
            for hp in range(KT):
                for nt in range(NT):
                    for mt in (hp, KT + hp):
                        if hp == 0 and nt == 0:
                            continue  # prologue
                        abunits.append(
                            (8 * hp + 2 * nt,
                             lambda mt=mt, nt=nt: unit_a(mt, nt))
                        )
            abunits.sort(key=lambda u: u[0])
            dunits = []
            for nt in range(NT):
                for mt in range(KT):
                    # window nt's last norms (iter 41+2nt) are emitted during
                    # iteration 42+2nt, after that iteration's filler slot
                    dunits.append(
                        (43 + 2 * nt, lambda nt=nt, mt=mt: unit_d(nt, mt))
                    )

            # ---- prologue ----
            for hp in range(KT):
                for jt in range(2):
                    nc.scalar.dma_start(
                        out=bias_sb[(hp, jt)][:],
                        in_=bT[hp, jt * 128 : jt * 128 + JROWS[jt], :],
                    )
            for k in range(KT):
                nc.gpsimd.dma_start(out=wpj_sb[k][:], in_=wpj[k])
            for b in range(2):
                for jt in range(2):
                    for n2 in range(2):
                        unit_b(b, jt, n2)
            unit_a(0, 0)
            unit_a(KT, 0)

            # ---- flat pipelined loop ----
            NITER = KT * BPC
            ai = di = 0
            for g in range(NITER + 1):
                # hard-due AB units must precede S[g]
                while ai < len(abunits) and abunits[ai][0] <= g:
                    abunits[ai][1]()
                    ai += 1
                if g < NITER:
                    emit_s(g)
                # soft top-up fillers between S[g] and AV[g-1]
                filled = 0
                while di < len(dunits) and dunits[di][0] <= g:
                    dunits[di][1]()
                    di += 1
                    filled += 1
                while filled < 2 and ai < len(abunits) and abunits[ai][0] <= g + 6:
                    abunits[ai][1]()
                    ai += 1
                    filled += 1
                if g >= 1:
                    emit_av(g - 1)
            while ai < len(abunits):
                abunits[ai][1]()
                ai += 1
            while di < len(dunits):
                dunits[di][1]()
                di += 1

    if finalize:
        nc.finalize()
    return nc


def _split8(a):
    """Error-compensated fp8 pair: a ~= hi + lo, each e4m3."""
    hi = a.astype(F8)
    lo = (a - hi.astype(np.float32)).astype(F8)
    return hi, lo


def _ktiles(a, nf):
    """(768, nf) -> (128, KT, nf) partition-major k-tiles."""
    return np.ascontiguousarray(a.reshape(KT, 128, nf).transpose(1, 0, 2))


def _host_prep(x, qkv_w, qkv_b, proj_w, proj_b, rel_table, log_temp, rel_index):
    """Build the per-core input maps (host-side layout prep only)."""
    x = np.asarray(x, np.float32)
    qkv_w = np.asarray(qkv_w, np.float32)
    qkv_b = np.asarray(qkv_b, np.float32)
    proj_w = np.asarray(proj_w, np.float32)
    rel_table = np.asarray(rel_table, np.float32)
    log_temp = np.asarray(log_temp, np.float32)
    rel_index = np.asarray(rel_index)

    temp = np.log1p(np.exp(log_temp.astype(np.float64))).astype(np.float32)
    alpha = (SCALE / temp).astype(np.float32)         # (H,) folded into q
    alpha_c = np.repeat(alpha, D)                     # (768,)

    # qk weights, host-scaled for fp8 range (SQ incl. alpha; SK plain),
    # split into hi/lo e4m3 pairs; hi-only and interleaved-cross layouts
    wqkT = qkv_w[0 : 2 * C].T.copy()                  # (768, 1536)
    wqkT[:, 0:C] *= alpha_c[None, :] * SQ
    wqkT[:, C : 2 * C] *= SK
    qhi, qlo = _split8(wqkT)
    qhi_t = _ktiles(qhi.astype(np.float32), 2 * C)
    qlo_t = _ktiles(qlo.astype(np.float32), 2 * C)
    wqk_hi_np = np.ascontiguousarray(
        qhi_t.reshape(128, KT // 2, 2, 2 * KT, 128).transpose(0, 3, 1, 2, 4)
    ).astype(F8)
    # cross weights LO-FIRST so the shared x cross buffer can stay HI-FIRST
    wqk_x_np = np.ascontiguousarray(
        np.stack([qlo_t, qhi_t], axis=2)
        .reshape(128, KT, 2, 2 * KT, 128)
        .transpose(0, 3, 1, 2, 4)
    ).astype(F8)

    # wv as fp8 hi/lo cross pairs, LO-FIRST (moving operand of V phase),
    # host-scaled by SVF out of the e4m3 subnormal range
    wvT = qkv_w[2 * C : 3 * C].T * SVF                # (768, 768)
    vhi, vlo = _split8(wvT)
    vhi_t = _ktiles(vhi.astype(np.float32), C)
    vlo_t = _ktiles(vlo.astype(np.float32), C)
    wvx = np.stack([vlo_t, vhi_t], axis=2)            # (128, KT, 2, C) lo-first
    wv_x_np = np.stack(
        [wvx[:, :, :, 0 : C // 2], wvx[:, :, :, C // 2 : C]], axis=0
    ).astype(F8)
    wpj_np = proj_w.T.reshape(KT, 128, C).astype(BF16)

    bq = qkv_b[0:C] * alpha_c
    bk = qkv_b[C : 2 * C]
    bqk_np = np.concatenate([bq, bk]).reshape(2 * KT, 128).T.copy().astype(np.float32)

    # multiplicative bias table: exp((relpos bias)/temp), diag -> 0, CLS -> 1,
    # transposed to (j, i); paired layout (KT, j, 2N)
    rpb = rel_table[rel_index]                        # (196, 196, H)
    bias = np.zeros((H, N, N), np.float32)
    bias[:, 1:, 1:] = rpb.transpose(2, 0, 1) / temp[:, None, None]
    ebias = np.exp(bias)
    idx = np.arange(1, N)
    ebias[:, idx, idx] = 0.0
    ebT = ebias.transpose(0, 2, 1)                    # (H, j, i)
    bT_np = (
        ebT.reshape(KT, 2, N, N).transpose(0, 2, 1, 3).reshape(KT, N, 2 * N)
    ).astype(BF16).copy()

    # window blocking: memory order [w0, w2, w1, w3], zero-padded to 128
    WOFF = (0, 197, 128, 325)
    WLEN = (128, 128, 69, 69)

    def _blocked(t, nunit):
        out = np.zeros((NT, 128, 4, nunit, 2, 128), np.float32)
        for nt in range(NT):
            for w in range(4):
                o = nt * TN + WOFF[w]
                out[nt, :, w, :, :, 0 : WLEN[w]] = t[:, :, :, o : o + WLEN[w]]
        return out.astype(F8)

    in_maps = []
    for c in range(NCORES):
        xc = x[c * BPC : (c + 1) * BPC].reshape(T, C).T  # (768, T)
        xhi, xlo = _split8(xc)
        xhi_t = _ktiles(xhi.astype(np.float32), T)      # (128, KT, T)
        xlo_t = _ktiles(xlo.astype(np.float32), T)
        xhi_p = xhi_t.reshape(128, KT // 2, 2, T)
        xx_p = np.stack([xhi_t, xlo_t], axis=2)         # (128, KT, 2, T)
        x_hi_np = _blocked(xhi_p, KT // 2)
        x_x_np = _blocked(xx_p, KT)
        in_maps.append(
            {
                "x_hi": x_hi_np,
                "x_x": x_x_np,
                "wv_x": wv_x_np,
                "wqk_hi": wqk_hi_np,
                "wqk_x": wqk_x_np,
                "wpj": wpj_np,
                "bT": bT_np,
                "bqk": bqk_np,
            }
        )
    return in_maps


def kernel(**inputs) -> np.ndarray:
    global LAST_RESULTS
    from concourse.bass_utils import run_bass_kernel_spmd

    if "nc" not in _CACHE:
        _CACHE["nc"] = _build()
    nc = _CACHE["nc"]

    in_maps = _host_prep(**inputs)
    try:
        res = run_bass_kernel_spmd(
            nc, in_maps, core_ids=list(range(NCORES)), trace=TRACE
        )
    except ModuleNotFoundError:
        res = run_bass_kernel_spmd(
            nc, in_maps, core_ids=list(range(NCORES)), trace=False
        )
    LAST_RESULTS = res

    # v-bias rides through attention unchanged (rows of attn sum to 1), so
    # its proj image folds into the constant output bias added here
    proj_b = np.asarray(inputs["proj_b"], np.float32)
    proj_w = np.asarray(inputs["proj_w"], np.float32)
    bv = np.asarray(inputs["qkv_b"], np.float32)[2 * C : 3 * C]
    b_eff = proj_b + proj_w @ bv
    outs = []
    for c in range(NCORES):
        oT = np.asarray(res.results[c]["outT"], np.float32).reshape(C, T)
        outs.append(oT.T.reshape(BPC, N, C))
    out = np.concatenate(outs, axis=0) + b_eff[None, None, :]
    return out.astype(np.float32)


# revision 7
# speedup vs baseline: 1.0608x; 1.0608x over previous
"""Trainium2 Bass kernel for nn_Attention_11055245820093.

Swin-style attention block: qkv proj -> per-head scaled dot-product attention
with 2D relative position bias (CLS zero-padded), per-head softplus temperature,
patch-diagonal mask -> proj.

Strategy: data-parallel over batch B=64 across 8 NeuronCores (8 batches/core).
All compute per core runs in a "transposed" layout (channels on partitions,
tokens on the free dim) so no on-device transposes are needed.

Numerics (validated at rel_err ~5e-3): projections in fp8e4m3 DoubleRow with
full error compensation (hi*hi pairs + interleaved cross terms, 9 DR
instructions per 768-contraction tile = 1.33x bf16); attention (QK^T, exp,
bias, AV, softmax divide) in bf16 -- every UNcompensated e4m3 activation
quantization alone costs ~2.5e-2 max-rel (over the 2e-2 gate), so fp8
attention is not affordable.

v3 performance structure (PE busy ~114us is the cost-model floor for this
algorithm; the work is eliminating PE idle):
  - ONE flat software-pipelined loop over (head-pair, batch): iteration g
    issues S[g] (QK^T), then PE "filler" units, then AV[g-1] -- AV runs one
    iteration behind so its exp/bias-mul input chain (ACT+Pool) has a full
    iteration of slack and PE never stalls on it.
  - PE filler units drawn from a static earliest-deadline schedule: V-proj
    units B[b] (due before AV uses V[b]), QK-proj subtiles (due before the
    head-pair's S reads that token window), and output-proj groups D(nt, mt)
    (ready once the last head-pair's norms for window nt completed).
  - bias multiply (e = exp(S) * ebias) runs on GPSIMD/Pool (proxy library
    tensor_tensor, SBUF-only) -- off the DVE critical path.
  - QK-proj PSUM uses token-order window offsets so each eviction is ONE
    contiguous [128, 394] op (the baseline's window-permuted layout needed
    two strided ops).
  - evictions alternate ACT/DVE to balance; outT DMAs round-robin across
    engine queues (they serialized on SP in the baseline, 3.6us tail).
"""

import os
import sys

sys.path.insert(0, "/opt/trn_rl_repo")
os.environ.setdefault("MYCRO_LOCAL_CACHE", "1")

import numpy as np
import ml_dtypes

BF16 = ml_dtypes.bfloat16
F8 = ml_dtypes.float8_e4m3fn

# Problem constants (hardcoded per contract)
B, N, C, H, D = 64, 197, 768, 12, 64
NCORES = 8
BPC = B // NCORES          # 8 batches per core
T = BPC * N                # 1576 tokens per core
KT = C // 128              # 6 contraction tiles of 128
NT = 4                     # token n-tiles
TN = T // NT               # 394 tokens per n-tile
SCALE = D ** -0.5
JROWS = (128, N - 128)     # 128, 69
N2 = 2 * N
SVF = 64.0                 # wv host-scale; ones column matches so the
                           # softmax divide cancels it exactly
SQ, SK = 256.0, 64.0

_CACHE = {}

TRACE = False
LAST_RESULTS = None


def _build(finalize=True):
    import concourse.bass as bass
    import concourse.tile as tile
    from concourse import bacc, library_config, mybir

    dt = mybir.dt
    f32, bf16, f8 = dt.float32, dt.bfloat16, dt.float8e4
    AF = mybir.ActivationFunctionType
    OP = mybir.AluOpType
    DR = mybir.MatmulPerfMode.DoubleRow

    nc = bacc.Bacc("TRN2", target_bir_lowering=False, debug=False)

    x_hi = nc.dram_tensor(
        "x_hi", [NT, 128, 4, KT // 2, 2, 128], f8, kind="ExternalInput"
    ).ap()
    x_x = nc.dram_tensor(
        "x_x", [NT, 128, 4, KT, 2, 128], f8, kind="ExternalInput"
    ).ap()
    wv_x = nc.dram_tensor(
        "wv_x", [2, 128, KT, 2, C // 2], f8, kind="ExternalInput"
    ).ap()
    wqk_hi = nc.dram_tensor(
        "wqk_hi", [128, 2 * KT, KT // 2, 2, 128], f8, kind="ExternalInput"
    ).ap()
    wqk_x = nc.dram_tensor(
        "wqk_x", [128, 2 * KT, KT, 2, 128], f8, kind="ExternalInput"
    ).ap()
    wpj = nc.dram_tensor("wpj", [KT, 128, C], bf16, kind="ExternalInput").ap()
    bT = nc.dram_tensor("bT", [KT, N, 2 * N], bf16, kind="ExternalInput").ap()
    bqk = nc.dram_tensor("bqk", [128, 2 * KT], f32, kind="ExternalInput").ap()
    outT = nc.dram_tensor("outT", [KT, 128, T], f32, kind="ExternalOutput").ap()

    # token-order psum window offsets: w index = xh memory order
    # [w0(tok 0:128), w2(tok 197:325), w1(tok 128:197), w3(tok 325:394)]
    WMO = ((0, 128), (197, 128), (128, 69), (325, 69))

    with tile.TileContext(nc) as tc:
        from contextlib import ExitStack

        with ExitStack() as ctx:
            nc.gpsimd.load_library(library_config.proxy)
            cp = ctx.enter_context(tc.tile_pool(name="consts", bufs=1))
            psA = ctx.enter_context(tc.tile_pool(name="psA", bufs=2, space="PSUM"))
            psC = ctx.enter_context(tc.tile_pool(name="psC", bufs=3, space="PSUM"))
            wp = ctx.enter_context(tc.tile_pool(name="work", bufs=2))

            # ---- persistent SBUF tiles; DMAs in consumption order ----
            xh_sb = cp.tile([128, NT, 4, KT // 2, 2, 128], f8, name="xh", tag="xh")
            xx_sb = cp.tile([128, NT, 4, KT, 2, 128], f8, name="xx", tag="xx")
            wvx_sb = cp.tile([128, 2, KT, 2, C // 2], f8, name="wvx", tag="wvx")
            wqkh_sb = cp.tile(
                [128, 2 * KT, KT // 2, 2, 128], f8, name="wqkh", tag="wqkh"
            )
            wqkx_sb = cp.tile(
                [128, 2 * KT, KT, 2, 128], f8, name="wqkx", tag="wqkx"
            )
            bqk_sb = cp.tile([128, 2 * KT], f32, name="bqk", tag="bqk")
            # startup-critical DMAs, spread so each consumer unblocks at
            # its need time: B prologue needs xh0/xx0/wvx; prologue A units
            # need only the mt=0 and mt=6 slices of wqk; later windows and
            # head-pairs stream in during the loop
            nc.gpsimd.dma_start(out=wvx_sb[:, 0], in_=wv_x[0])
            nc.gpsimd.dma_start(out=wvx_sb[:, 1], in_=wv_x[1])
            nc.scalar.dma_start(out=xh_sb[:, 0], in_=x_hi[0])
            nc.scalar.dma_start(out=xx_sb[:, 0], in_=x_x[0])
            for m in (0, KT):
                nc.gpsimd.dma_start(out=wqkh_sb[:, m], in_=wqk_hi[:, m])
                nc.gpsimd.dma_start(out=wqkx_sb[:, m], in_=wqk_x[:, m])
            nc.scalar.dma_start(out=bqk_sb[:], in_=bqk[:])
            nc.scalar.dma_start(out=xh_sb[:, 1], in_=x_hi[1])
            nc.scalar.dma_start(out=xx_sb[:, 1], in_=x_x[1])
            for m in (1, KT + 1):
                nc.gpsimd.dma_start(out=wqkh_sb[:, m], in_=wqk_hi[:, m])
                nc.gpsimd.dma_start(out=wqkx_sb[:, m], in_=wqk_x[:, m])
            for nt in range(2, NT):
                nc.sync.dma_start(out=xh_sb[:, nt], in_=x_hi[nt])
                nc.sync.dma_start(out=xx_sb[:, nt], in_=x_x[nt])
            for m in range(2 * KT):
                if m % KT in (0, 1):
                    continue
                nc.sync.dma_start(out=wqkh_sb[:, m], in_=wqk_hi[:, m])
                nc.sync.dma_start(out=wqkx_sb[:, m], in_=wqk_x[:, m])
            bias_sb = {}
            for hp in range(KT):
                for jt, rows in enumerate(JROWS):
                    bias_sb[(hp, jt)] = cp.tile(
                        [rows, N2], bf16, name=f"bias{hp}_{jt}", tag=f"bias{hp}_{jt}"
                    )
            wpj_sb = [
                cp.tile([128, C], bf16, name=f"wpj{k}", tag=f"wpj{k}")
                for k in range(KT)
            ]

            # qk tiles: Q (mt 0..5), K (mt 6..11)
            qk_sb = [
                cp.tile([128, T], bf16, name=f"qk{m}", tag=f"qk{m}")
                for m in range(2 * KT)
            ]
            # V per (batch, jt): (rows, 12 heads, 65) -- 64 V cols + ones col
            v_sb = {}
            for b in range(BPC):
                for jt, rows in enumerate(JROWS):
                    t_ = cp.tile(
                        [rows, H, D + 1], bf16, name=f"v{b}_{jt}", tag=f"v{b}_{jt}"
                    )
                    nc.vector.memset(t_[:, :, D : D + 1], SVF)
                    v_sb[(b, jt)] = t_
            attn_sb = [
                cp.tile([128, T], bf16, name=f"at{m}", tag=f"at{m}") for m in range(KT)
            ]

            evict_flip = [0]

            def evict_engine():
                evict_flip[0] += 1
                return nc.vector if evict_flip[0] % 2 == 0 else nc.scalar

            # ---- filler unit emitters (pure PE work + one eviction) ----
            def unit_b(b, jt, n2):
                """V-proj quarter: one psum group -> v_sb[(b, jt)] slice."""
                rows = JROWS[jt]
                ntb = b // 2
                wpos = 2 * jt + (b % 2)
                psv = psA.tile([128, 512], f32, tag="psA")
                for p in range(KT // 2):
                    nc.tensor.matmul(
                        psv[0:128, 0 : C // 2],
                        xh_sb[:, ntb, wpos, p, :, :],
                        wvx_sb[:, n2, 2 * p : 2 * p + 2, 1, :],
                        start=(p == 0),
                        stop=False,
                        perf_mode=DR,
                    )
                for k in range(KT):
                    nc.tensor.matmul(
                        psv[0:128, 0 : C // 2],
                        xx_sb[:, ntb, wpos, k, :, :],
                        wvx_sb[:, n2, k, :, :],
                        start=False,
                        stop=(k == KT - 1),
                        perf_mode=DR,
                    )
                eng = evict_engine()
                dst = v_sb[(b, jt)][0:rows, n2 * KT : (n2 + 1) * KT, 0:D]
                src = psv[0:rows, 0 : C // 2].rearrange("p (h d) -> p h d", h=KT)
                if eng is nc.vector:
                    nc.vector.tensor_copy(dst, src)
                else:
                    nc.scalar.activation(dst, src, AF.Copy)

            def unit_a(mt, nt):
                """QK-proj subtile: one token window of Q or K tile mt."""
                inv_s = (1.0 / SQ) if mt < KT else (1.0 / SK)
                ps = psA.tile([128, 512], f32, tag="psA")
                first = True
                for p in range(KT // 2):
                    for w, (o, gl) in enumerate(WMO):
                        nc.tensor.matmul(
                            ps[:, o : o + gl],
                            wqkh_sb[:, mt, p, :, :],
                            xh_sb[:, nt, w, p, :, 0:gl],
                            start=first,
                            stop=False,
                            perf_mode=DR,
                        )
                        first = False
                for k in range(KT):
                    for w, (o, gl) in enumerate(WMO):
                        nc.tensor.matmul(
                            ps[:, o : o + gl],
                            wqkx_sb[:, mt, k, :, :],
                            xx_sb[:, nt, w, k, :, 0:gl],
                            start=False,
                            stop=(k == KT - 1 and w == 3),
                            perf_mode=DR,
                        )
                # token-order psum -> single contiguous eviction
                dst = qk_sb[mt][:, nt * TN : (nt + 1) * TN]
                if evict_engine() is nc.vector:
                    nc.vector.tensor_scalar(
                        dst, ps[:, 0:TN], inv_s, bqk_sb[:, mt : mt + 1],
                        OP.mult, OP.add,
                    )
                else:
                    nc.scalar.activation(
                        dst, ps[:, 0:TN], AF.Identity,
                        bias=bqk_sb[:, mt : mt + 1], scale=inv_s,
                    )

            dq = [0]

            def unit_d(nt, mt):
                """output-proj group: one (window, out-tile) -> outT DMA."""
                ps = psA.tile([128, 512], f32, tag="psA")
                for k in range(KT):
                    nc.tensor.matmul(
                        ps[:, 0:TN],
                        wpj_sb[k][:, mt * 128 : (mt + 1) * 128],
                        attn_sb[k][:, nt * TN : (nt + 1) * TN],
                        start=(k == 0),
                        stop=(k == KT - 1),
                    )
                ot = wp.tile([128, TN], f32, tag="ot", bufs=3)
                if evict_engine() is nc.vector:
                    nc.vector.tensor_copy(ot[:], ps[:, 0:TN])
                else:
                    nc.scalar.activation(ot[:], ps[:, 0:TN], AF.Copy)
                q = (nc.sync, nc.gpsimd)[dq[0] % 2]
                dq[0] += 1
                q.dma_start(out=outT[mt, :, nt * TN : (nt + 1) * TN], in_=ot[:])

            # ---- attention pieces ----
            s_tiles = {}

            def emit_s(g):
                """S^T matmuls + exp + Pool bias-mul for iteration g."""
                hp, b = divmod(g, BPC)
                e2 = wp.tile([128, 2, 2, N], bf16, tag="e2", bufs=3)
                for jt, rows in enumerate(JROWS):
                    ps = psC.tile([128, 2, 512], f32, tag="psC")
                    for hh in range(2):
                        base = 64 * hh
                        nc.tensor.matmul(
                            ps[0:rows, hh, 0:N],
                            qk_sb[KT + hp][
                                base : base + 64,
                                b * N + jt * 128 : b * N + jt * 128 + rows,
                            ],
                            qk_sb[hp][base : base + 64, b * N : (b + 1) * N],
                            start=True,
                            stop=True,
                        )
                    eu = wp.tile([128, 2, N], bf16, tag=f"eu{jt}", bufs=3)
                    nc.scalar.activation(
                        eu[0:rows, :, :], ps[0:rows, :, 0:N], AF.Exp
                    )
                    # multiplicative rel-pos bias (exp'd on host) on Pool
                    nc.gpsimd.tensor_mul(
                        e2[0:rows, :, jt, :],
                        eu[0:rows, :, :],
                        bias_sb[(hp, jt)][0:rows, :].rearrange(
                            "p (g n) -> p g n", g=2
                        ),
                    )
                s_tiles[g] = e2

            def emit_av(g):
                """AV + softmax normalize for iteration g (runs at g+1)."""
                hp, b = divmod(g, BPC)
                e2 = s_tiles.pop(g)
                po = psC.tile([128, 2, 512], f32, tag="psC")
                for hh in range(2):
                    h = 2 * hp + hh
                    for jt, rows in enumerate(JROWS):
                        nc.tensor.matmul(
                            po[0 : D + 1, hh, 0:N],
                            v_sb[(b, jt)][0:rows, h, 0 : D + 1],
                            e2[0:rows, hh, jt, :],
                            start=(jt == 0),
                            stop=(jt == 1),
                        )
                r2 = wp.tile([1, 2, N], bf16, tag="r2", bufs=3)
                with nc.allow_low_precision(
                    reason="softmax denom reciprocal in bf16"
                ):
                    nc.vector.reciprocal(r2[:, :, :], po[D : D + 1, :, 0:N])
                rb = wp.tile([128, N2], bf16, tag="rb", bufs=3)
                nc.gpsimd.partition_broadcast(rb[:, :], r2[:, :, :])
                nc.vector.tensor_mul(
                    attn_sb[hp][0:D, b * N : (b + 1) * N],
                    po[0:D, 0, 0:N],
                    rb[0:D, 0:N],
                )
                nc.vector.tensor_mul(
                    attn_sb[hp][D : 2 * D, b * N : (b + 1) * N],
                    po[0:D, 1, 0:N],
                    rb[D : 2 * D, N:N2],
                )

            # ---- static filler schedule ----
            # AB units have HARD deadlines (due = first iteration whose S or
            # AV reads their output; emitting later would cycle the in-order
            # PE queue through an ACT/DVE eviction that sits behind stalled
            # work -> deadlock).  D units instead have a READY iteration
            # (earliest emission keeping attn writes ahead in PE order).
            abunits = []
            for b in range(2, BPC):
                for jt in range(2):
                    for n2 in range(2):
                        # V[b] consumed by AV[b] emitted at iteration b+1
                        abunits.append(
                            (b, lambda b=b, jt=jt, n2=n2: unit_b(b, jt, n2))
                        )
            for hp in range(KT):
                for nt in range(NT):
                    for mt in (hp, KT + hp):
                        if hp == 0 and nt == 0:
                            continue  # prologue
                        abunits.append(
                            (8 * hp + 2 * nt,
                             lambda mt=mt, nt=nt: unit_a(mt, nt))
                        )
            abunits.sort(key=lambda u: u[0])
            dunits = []
            for nt in range(NT):
                for mt in range(KT):
                    # window nt's last norms (iter 41+2nt) are emitted during
                    # iteration 42+2nt, after that iteration's filler slot
                    dunits.append(
                        (43 + 2 * nt, lambda nt=nt, mt=mt: unit_d(nt, mt))
                    )

            # ---- prologue ----
            for hp in range(KT):
                for jt in range(2):
                    (nc.gpsimd if hp < 2 else nc.sync).dma_start(
                        out=bias_sb[(hp, jt)][:],
                        in_=bT[hp, jt * 128 : jt * 128 + JROWS[jt], :],
                    )
            for k in range(KT):
                nc.sync.dma_start(out=wpj_sb[k][:], in_=wpj[k])
            for b in range(2):
                for jt in range(2):
                    for n2 in range(2):
                        unit_b(b, jt, n2)
            unit_a(0, 0)
            unit_a(KT, 0)

            # ---- flat pipelined loop ----
            NITER = KT * BPC
            ai = di = 0
            for g in range(NITER + 1):
                # hard-due AB units must precede S[g]
                while ai < len(abunits) and abunits[ai][0] <= g:
                    abunits[ai][1]()
                    ai += 1
                if g < NITER:
                    emit_s(g)
                # soft top-up fillers between S[g] and AV[g-1]
                filled = 0
                while di < len(dunits) and dunits[di][0] <= g:
                    dunits[di][1]()
                    di += 1
                    filled += 1
                while filled < 2 and ai < len(abunits) and abunits[ai][0] <= g + 6:
                    abunits[ai][1]()
                    ai += 1
                    filled += 1
                if g >= 1:
                    emit_av(g - 1)
            while ai < len(abunits):
                abunits[ai][1]()
                ai += 1
            while di < len(dunits):
                dunits[di][1]()
                di += 1

    if finalize:
        nc.finalize()
    return nc


def _split8(a):
    """Error-compensated fp8 pair: a ~= hi + lo, each e4m3."""
    hi = a.astype(F8)
    lo = (a - hi.astype(np.float32)).astype(F8)
    return hi, lo


def _ktiles(a, nf):
    """(768, nf) -> (128, KT, nf) partition-major k-tiles."""
    return np.ascontiguousarray(a.reshape(KT, 128, nf).transpose(1, 0, 2))


def _host_prep(x, qkv_w, qkv_b, proj_w, proj_b, rel_table, log_temp, rel_index):
    """Build the per-core input maps (host-side layout prep only)."""
    x = np.asarray(x, np.float32)
    qkv_w = np.asarray(qkv_w, np.float32)
    qkv_b = np.asarray(qkv_b, np.float32)
    proj_w = np.asarray(proj_w, np.float32)
    rel_table = np.asarray(rel_table, np.float32)
    log_temp = np.asarray(log_temp, np.float32)
    rel_index = np.asarray(rel_index)

    temp = np.log1p(np.exp(log_temp.astype(np.float64))).astype(np.float32)
    alpha = (SCALE / temp).astype(np.float32)         # (H,) folded into q
    alpha_c = np.repeat(alpha, D)                     # (768,)

    # qk weights, host-scaled for fp8 range (SQ incl. alpha; SK plain),
    # split into hi/lo e4m3 pairs; hi-only and interleaved-cross layouts
    wqkT = qkv_w[0 : 2 * C].T.copy()                  # (768, 1536)
    wqkT[:, 0:C] *= alpha_c[None, :] * SQ
    wqkT[:, C : 2 * C] *= SK
    qhi, qlo = _split8(wqkT)
    qhi_t = _ktiles(qhi.astype(np.float32), 2 * C)
    qlo_t = _ktiles(qlo.astype(np.float32), 2 * C)
    wqk_hi_np = np.ascontiguousarray(
        qhi_t.reshape(128, KT // 2, 2, 2 * KT, 128).transpose(0, 3, 1, 2, 4)
    ).astype(F8)
    # cross weights LO-FIRST so the shared x cross buffer can stay HI-FIRST
    wqk_x_np = np.ascontiguousarray(
        np.stack([qlo_t, qhi_t], axis=2)
        .reshape(128, KT, 2, 2 * KT, 128)
        .transpose(0, 3, 1, 2, 4)
    ).astype(F8)

    # wv as fp8 hi/lo cross pairs, LO-FIRST (moving operand of V phase),
    # host-scaled by SVF out of the e4m3 subnormal range
    wvT = qkv_w[2 * C : 3 * C].T * SVF                # (768, 768)
    vhi, vlo = _split8(wvT)
    vhi_t = _ktiles(vhi.astype(np.float32), C)
    vlo_t = _ktiles(vlo.astype(np.float32), C)
    wvx = np.stack([vlo_t, vhi_t], axis=2)            # (128, KT, 2, C) lo-first
    wv_x_np = np.stack(
        [wvx[:, :, :, 0 : C // 2], wvx[:, :, :, C // 2 : C]], axis=0
    ).astype(F8)
    wpj_np = proj_w.T.reshape(KT, 128, C).astype(BF16)

    bq = qkv_b[0:C] * alpha_c
    bk = qkv_b[C : 2 * C]
    bqk_np = np.concatenate([bq, bk]).reshape(2 * KT, 128).T.copy().astype(np.float32)

    # multiplicative bias table: exp((relpos bias)/temp), diag -> 0, CLS -> 1,
    # transposed to (j, i); paired layout (KT, j, 2N)
    rpb = rel_table[rel_index]                        # (196, 196, H)
    bias = np.zeros((H, N, N), np.float32)
    bias[:, 1:, 1:] = rpb.transpose(2, 0, 1) / temp[:, None, None]
    ebias = np.exp(bias)
    idx = np.arange(1, N)
    ebias[:, idx, idx] = 0.0
    ebT = ebias.transpose(0, 2, 1)                    # (H, j, i)
    bT_np = (
        ebT.reshape(KT, 2, N, N).transpose(0, 2, 1, 3).reshape(KT, N, 2 * N)
    ).astype(BF16).copy()

    # window blocking: memory order [w0, w2, w1, w3], zero-padded to 128
    WOFF = (0, 197, 128, 325)
    WLEN = (128, 128, 69, 69)

    def _blocked(t, nunit):
        out = np.zeros((NT, 128, 4, nunit, 2, 128), np.float32)
        for nt in range(NT):
            for w in range(4):
                o = nt * TN + WOFF[w]
                out[nt, :, w, :, :, 0 : WLEN[w]] = t[:, :, :, o : o + WLEN[w]]
        return out.astype(F8)

    in_maps = []
    for c in range(NCORES):
        xc = x[c * BPC : (c + 1) * BPC].reshape(T, C).T  # (768, T)
        xhi, xlo = _split8(xc)
        xhi_t = _ktiles(xhi.astype(np.float32), T)      # (128, KT, T)
        xlo_t = _ktiles(xlo.astype(np.float32), T)
        xhi_p = xhi_t.reshape(128, KT // 2, 2, T)
        xx_p = np.stack([xhi_t, xlo_t], axis=2)         # (128, KT, 2, T)
        x_hi_np = _blocked(xhi_p, KT // 2)
        x_x_np = _blocked(xx_p, KT)
        in_maps.append(
            {
                "x_hi": x_hi_np,
                "x_x": x_x_np,
                "wv_x": wv_x_np,
                "wqk_hi": wqk_hi_np,
                "wqk_x": wqk_x_np,
                "wpj": wpj_np,
                "bT": bT_np,
                "bqk": bqk_np,
            }
        )
    return in_maps


def kernel(**inputs) -> np.ndarray:
    global LAST_RESULTS
    from concourse.bass_utils import run_bass_kernel_spmd

    if "nc" not in _CACHE:
        _CACHE["nc"] = _build()
    nc = _CACHE["nc"]

    in_maps = _host_prep(**inputs)
    try:
        res = run_bass_kernel_spmd(
            nc, in_maps, core_ids=list(range(NCORES)), trace=TRACE
        )
    except ModuleNotFoundError:
        res = run_bass_kernel_spmd(
            nc, in_maps, core_ids=list(range(NCORES)), trace=False
        )
    LAST_RESULTS = res

    # v-bias rides through attention unchanged (rows of attn sum to 1), so
    # its proj image folds into the constant output bias added here
    proj_b = np.asarray(inputs["proj_b"], np.float32)
    proj_w = np.asarray(inputs["proj_w"], np.float32)
    bv = np.asarray(inputs["qkv_b"], np.float32)[2 * C : 3 * C]
    b_eff = proj_b + proj_w @ bv
    outs = []
    for c in range(NCORES):
        oT = np.asarray(res.results[c]["outT"], np.float32).reshape(C, T)
        outs.append(oT.T.reshape(BPC, N, C))
    out = np.concatenate(outs, axis=0) + b_eff[None, None, :]
    return out.astype(np.float32)


# revision 8
# speedup vs baseline: 1.0671x; 1.0059x over previous
"""Trainium2 Bass kernel for nn_Attention_11055245820093.

Swin-style attention block: qkv proj -> per-head scaled dot-product attention
with 2D relative position bias (CLS zero-padded), per-head softplus temperature,
patch-diagonal mask -> proj.

Strategy: data-parallel over batch B=64 across 8 NeuronCores (8 batches/core).
All compute per core runs in a "transposed" layout (channels on partitions,
tokens on the free dim) so no on-device transposes are needed.

Numerics (validated at rel_err ~5e-3): projections in fp8e4m3 DoubleRow with
full error compensation (hi*hi pairs + interleaved cross terms, 9 DR
instructions per 768-contraction tile = 1.33x bf16); attention (QK^T, exp,
bias, AV, softmax divide) in bf16 -- every UNcompensated e4m3 activation
quantization alone costs ~2.5e-2 max-rel (over the 2e-2 gate), so fp8
attention is not affordable.

v3 performance structure (PE busy ~114us is the cost-model floor for this
algorithm; the work is eliminating PE idle):
  - ONE flat software-pipelined loop over (head-pair, batch): iteration g
    issues S[g] (QK^T), then PE "filler" units, then AV[g-1] -- AV runs one
    iteration behind so its exp/bias-mul input chain (ACT+Pool) has a full
    iteration of slack and PE never stalls on it.
  - PE filler units drawn from a static earliest-deadline schedule: V-proj
    units B[b] (due before AV uses V[b]), QK-proj subtiles (due before the
    head-pair's S reads that token window), and output-proj groups D(nt, mt)
    (ready once the last head-pair's norms for window nt completed).
  - bias multiply (e = exp(S) * ebias) runs on GPSIMD/Pool (proxy library
    tensor_tensor, SBUF-only) -- off the DVE critical path.
  - QK-proj PSUM uses token-order window offsets so each eviction is ONE
    contiguous [128, 394] op (the baseline's window-permuted layout needed
    two strided ops).
  - evictions alternate ACT/DVE to balance; outT DMAs round-robin across
    engine queues (they serialized on SP in the baseline, 3.6us tail).
"""

import os
import sys

sys.path.insert(0, "/opt/trn_rl_repo")
os.environ.setdefault("MYCRO_LOCAL_CACHE", "1")

import numpy as np
import ml_dtypes

BF16 = ml_dtypes.bfloat16
F8 = ml_dtypes.float8_e4m3fn

# Problem constants (hardcoded per contract)
B, N, C, H, D = 64, 197, 768, 12, 64
NCORES = 8
BPC = B // NCORES          # 8 batches per core
T = BPC * N                # 1576 tokens per core
KT = C // 128              # 6 contraction tiles of 128
NT = 4                     # token n-tiles
TN = T // NT               # 394 tokens per n-tile
SCALE = D ** -0.5
JROWS = (128, N - 128)     # 128, 69
N2 = 2 * N
SVF = 64.0                 # wv host-scale; ones column matches so the
                           # softmax divide cancels it exactly
SQ, SK = 256.0, 64.0

_CACHE = {}

TRACE = False
LAST_RESULTS = None


def _build(finalize=True):
    import concourse.bass as bass
    import concourse.tile as tile
    from concourse import bacc, library_config, mybir

    dt = mybir.dt
    f32, bf16, f8 = dt.float32, dt.bfloat16, dt.float8e4
    AF = mybir.ActivationFunctionType
    OP = mybir.AluOpType
    DR = mybir.MatmulPerfMode.DoubleRow

    nc = bacc.Bacc("TRN2", target_bir_lowering=False, debug=False)

    x_hi = nc.dram_tensor(
        "x_hi", [NT, 128, 4, KT // 2, 2, 128], f8, kind="ExternalInput"
    ).ap()
    x_x = nc.dram_tensor(
        "x_x", [NT, 128, 4, KT, 2, 128], f8, kind="ExternalInput"
    ).ap()
    wv_x = nc.dram_tensor(
        "wv_x", [2, 128, KT, 2, C // 2], f8, kind="ExternalInput"
    ).ap()
    wqk_hi = nc.dram_tensor(
        "wqk_hi", [128, 2 * KT, KT // 2, 2, 128], f8, kind="ExternalInput"
    ).ap()
    wqk_x = nc.dram_tensor(
        "wqk_x", [128, 2 * KT, KT, 2, 128], f8, kind="ExternalInput"
    ).ap()
    wpj = nc.dram_tensor("wpj", [KT, 128, C], bf16, kind="ExternalInput").ap()
    bT = nc.dram_tensor("bT", [KT, N, 2 * N], bf16, kind="ExternalInput").ap()
    bqk = nc.dram_tensor("bqk", [128, 2 * KT], f32, kind="ExternalInput").ap()
    outT = nc.dram_tensor("outT", [KT, 128, T], f32, kind="ExternalOutput").ap()

    # token-order psum window offsets: w index = xh memory order
    # [w0(tok 0:128), w2(tok 197:325), w1(tok 128:197), w3(tok 325:394)]
    WMO = ((0, 128), (197, 128), (128, 69), (325, 69))

    with tile.TileContext(nc) as tc:
        from contextlib import ExitStack

        with ExitStack() as ctx:
            nc.gpsimd.load_library(library_config.proxy)
            cp = ctx.enter_context(tc.tile_pool(name="consts", bufs=1))
            psA = ctx.enter_context(tc.tile_pool(name="psA", bufs=2, space="PSUM"))
            psC = ctx.enter_context(tc.tile_pool(name="psC", bufs=3, space="PSUM"))
            wp = ctx.enter_context(tc.tile_pool(name="work", bufs=2))

            # ---- persistent SBUF tiles; DMAs in consumption order ----
            xh_sb = cp.tile([128, NT, 4, KT // 2, 2, 128], f8, name="xh", tag="xh")
            xx_sb = cp.tile([128, NT, 4, KT, 2, 128], f8, name="xx", tag="xx")
            wvx_sb = cp.tile([128, 2, KT, 2, C // 2], f8, name="wvx", tag="wvx")
            wqkh_sb = cp.tile(
                [128, 2 * KT, KT // 2, 2, 128], f8, name="wqkh", tag="wqkh"
            )
            wqkx_sb = cp.tile(
                [128, 2 * KT, KT, 2, 128], f8, name="wqkx", tag="wqkx"
            )
            bqk_sb = cp.tile([128, 2 * KT], f32, name="bqk", tag="bqk")
            # startup-critical DMAs, spread so each consumer unblocks at
            # its need time: B prologue needs xh0/xx0/wvx; prologue A units
            # need only the mt=0 and mt=6 slices of wqk; later windows and
            # head-pairs stream in during the loop
            nc.gpsimd.dma_start(out=wvx_sb[:, 0], in_=wv_x[0])
            nc.gpsimd.dma_start(out=wvx_sb[:, 1], in_=wv_x[1])
            nc.scalar.dma_start(out=xh_sb[:, 0], in_=x_hi[0])
            nc.scalar.dma_start(out=xx_sb[:, 0], in_=x_x[0])
            # wqk mt-slices in hp-need order (both Q and K of pair hp are
            # needed by iteration hp), alternating gpsimd/sync
            for hp in range(KT):
                qa = nc.gpsimd if hp % 2 == 0 else nc.sync
                for m in (hp, KT + hp):
                    qa.dma_start(out=wqkh_sb[:, m], in_=wqk_hi[:, m])
                    qa.dma_start(out=wqkx_sb[:, m], in_=wqk_x[:, m])
            nc.scalar.dma_start(out=bqk_sb[:], in_=bqk[:])
            for nt in range(1, NT):
                nc.sync.dma_start(out=xh_sb[:, nt], in_=x_hi[nt])
                nc.sync.dma_start(out=xx_sb[:, nt], in_=x_x[nt])
            bias_sb = {}
            for hp in range(KT):
                for jt, rows in enumerate(JROWS):
                    bias_sb[(hp, jt)] = cp.tile(
                        [rows, N2], bf16, name=f"bias{hp}_{jt}", tag=f"bias{hp}_{jt}"
                    )
            wpj_sb = [
                cp.tile([128, C], bf16, name=f"wpj{k}", tag=f"wpj{k}")
                for k in range(KT)
            ]

            # qk tiles: Q (mt 0..5), K (mt 6..11)
            qk_sb = [
                cp.tile([128, T], bf16, name=f"qk{m}", tag=f"qk{m}")
                for m in range(2 * KT)
            ]
            # V per (batch, jt): (rows, 12 heads, 65) -- 64 V cols + ones col
            v_sb = {}
            for b in range(BPC):
                for jt, rows in enumerate(JROWS):
                    t_ = cp.tile(
                        [rows, H, D + 1], bf16, name=f"v{b}_{jt}", tag=f"v{b}_{jt}"
                    )
                    nc.vector.memset(t_[:, :, D : D + 1], SVF)
                    v_sb[(b, jt)] = t_
            attn_sb = [
                cp.tile([128, T], bf16, name=f"at{m}", tag=f"at{m}") for m in range(KT)
            ]

            evict_flip = [0]

            def evict_engine():
                evict_flip[0] += 1
                return nc.vector if evict_flip[0] % 2 == 0 else nc.scalar

            # ---- filler unit emitters (pure PE work + one eviction) ----
            def unit_b(b, jt, n2):
                """V-proj quarter: one psum group -> v_sb[(b, jt)] slice."""
                rows = JROWS[jt]
                ntb = b // 2
                wpos = 2 * jt + (b % 2)
                psv = psA.tile([128, 512], f32, tag="psA")
                for p in range(KT // 2):
                    nc.tensor.matmul(
                        psv[0:128, 0 : C // 2],
                        xh_sb[:, ntb, wpos, p, :, :],
                        wvx_sb[:, n2, 2 * p : 2 * p + 2, 1, :],
                        start=(p == 0),
                        stop=False,
                        perf_mode=DR,
                    )
                for k in range(KT):
                    nc.tensor.matmul(
                        psv[0:128, 0 : C // 2],
                        xx_sb[:, ntb, wpos, k, :, :],
                        wvx_sb[:, n2, k, :, :],
                        start=False,
                        stop=(k == KT - 1),
                        perf_mode=DR,
                    )
                eng = evict_engine()
                dst = v_sb[(b, jt)][0:rows, n2 * KT : (n2 + 1) * KT, 0:D]
                src = psv[0:rows, 0 : C // 2].rearrange("p (h d) -> p h d", h=KT)
                if eng is nc.vector:
                    nc.vector.tensor_copy(dst, src)
                else:
                    nc.scalar.activation(dst, src, AF.Copy)

            def unit_a(mt, nt):
                """QK-proj subtile: one token window of Q or K tile mt."""
                inv_s = (1.0 / SQ) if mt < KT else (1.0 / SK)
                ps = psA.tile([128, 512], f32, tag="psA")
                first = True
                for p in range(KT // 2):
                    for w, (o, gl) in enumerate(WMO):
                        nc.tensor.matmul(
                            ps[:, o : o + gl],
                            wqkh_sb[:, mt, p, :, :],
                            xh_sb[:, nt, w, p, :, 0:gl],
                            start=first,
                            stop=False,
                            perf_mode=DR,
                        )
                        first = False
                for k in range(KT):
                    for w, (o, gl) in enumerate(WMO):
                        nc.tensor.matmul(
                            ps[:, o : o + gl],
                            wqkx_sb[:, mt, k, :, :],
                            xx_sb[:, nt, w, k, :, 0:gl],
                            start=False,
                            stop=(k == KT - 1 and w == 3),
                            perf_mode=DR,
                        )
                # token-order psum -> single contiguous eviction
                dst = qk_sb[mt][:, nt * TN : (nt + 1) * TN]
                if evict_engine() is nc.vector:
                    nc.vector.tensor_scalar(
                        dst, ps[:, 0:TN], inv_s, bqk_sb[:, mt : mt + 1],
                        OP.mult, OP.add,
                    )
                else:
                    nc.scalar.activation(
                        dst, ps[:, 0:TN], AF.Identity,
                        bias=bqk_sb[:, mt : mt + 1], scale=inv_s,
                    )

            dq = [0]

            def unit_d(nt, mt):
                """output-proj group: one (window, out-tile) -> outT DMA."""
                ps = psA.tile([128, 512], f32, tag="psA")
                for k in range(KT):
                    nc.tensor.matmul(
                        ps[:, 0:TN],
                        wpj_sb[k][:, mt * 128 : (mt + 1) * 128],
                        attn_sb[k][:, nt * TN : (nt + 1) * TN],
                        start=(k == 0),
                        stop=(k == KT - 1),
                    )
                ot = wp.tile([128, TN], f32, tag="ot", bufs=3)
                if evict_engine() is nc.vector:
                    nc.vector.tensor_copy(ot[:], ps[:, 0:TN])
                else:
                    nc.scalar.activation(ot[:], ps[:, 0:TN], AF.Copy)
                q = (nc.sync, nc.gpsimd)[dq[0] % 2]
                dq[0] += 1
                q.dma_start(out=outT[mt, :, nt * TN : (nt + 1) * TN], in_=ot[:])

            # ---- attention pieces ----
            s_tiles = {}

            def emit_s(g):
                """S^T matmuls + exp + Pool bias-mul for iteration g."""
                b, hp = divmod(g, KT)
                e2 = wp.tile([128, 2, 2, N], bf16, tag="e2", bufs=3)
                for jt, rows in enumerate(JROWS):
                    ps = psC.tile([128, 2, 512], f32, tag="psC")
                    for hh in range(2):
                        base = 64 * hh
                        nc.tensor.matmul(
                            ps[0:rows, hh, 0:N],
                            qk_sb[KT + hp][
                                base : base + 64,
                                b * N + jt * 128 : b * N + jt * 128 + rows,
                            ],
                            qk_sb[hp][base : base + 64, b * N : (b + 1) * N],
                            start=True,
                            stop=True,
                        )
                    eu = wp.tile([128, 2, N], bf16, tag=f"eu{jt}", bufs=3)
                    nc.scalar.activation(
                        eu[0:rows, :, :], ps[0:rows, :, 0:N], AF.Exp
                    )
                    # multiplicative rel-pos bias (exp'd on host) on Pool
                    nc.gpsimd.tensor_mul(
                        e2[0:rows, :, jt, :],
                        eu[0:rows, :, :],
                        bias_sb[(hp, jt)][0:rows, :].rearrange(
                            "p (g n) -> p g n", g=2
                        ),
                    )
                s_tiles[g] = e2

            def emit_av(g):
                """AV + softmax normalize for iteration g (runs at g+1)."""
                b, hp = divmod(g, KT)
                e2 = s_tiles.pop(g)
                po = psC.tile([128, 2, 512], f32, tag="psC")
                for hh in range(2):
                    h = 2 * hp + hh
                    for jt, rows in enumerate(JROWS):
                        nc.tensor.matmul(
                            po[0 : D + 1, hh, 0:N],
                            v_sb[(b, jt)][0:rows, h, 0 : D + 1],
                            e2[0:rows, hh, jt, :],
                            start=(jt == 0),
                            stop=(jt == 1),
                        )
                r2 = wp.tile([1, 2, N], bf16, tag="r2", bufs=3)
                with nc.allow_low_precision(
                    reason="softmax denom reciprocal in bf16"
                ):
                    nc.vector.reciprocal(r2[:, :, :], po[D : D + 1, :, 0:N])
                rb = wp.tile([128, N2], bf16, tag="rb", bufs=3)
                nc.gpsimd.partition_broadcast(rb[:, :], r2[:, :, :])
                nc.vector.tensor_mul(
                    attn_sb[hp][0:D, b * N : (b + 1) * N],
                    po[0:D, 0, 0:N],
                    rb[0:D, 0:N],
                )
                nc.vector.tensor_mul(
                    attn_sb[hp][D : 2 * D, b * N : (b + 1) * N],
                    po[0:D, 1, 0:N],
                    rb[D : 2 * D, N:N2],
                )

            # ---- static filler schedule ----
            # AB units have HARD deadlines (due = first iteration whose S or
            # AV reads their output; emitting later would cycle the in-order
            # PE queue through an ACT/DVE eviction that sits behind stalled
            # work -> deadlock).  D units instead have a READY iteration
            # (earliest emission keeping attn writes ahead in PE order).
            abunits = []
            for b in range(1, BPC):
                for jt in range(2):
                    for n2 in range(2):
                        # V[b] consumed by AV[b, hp0] emitted at iter 6b+1
                        abunits.append(
                            (max(0, KT * b - 1),
                             lambda b=b, jt=jt, n2=n2: unit_b(b, jt, n2))
                        )
            for hp in range(KT):
                for nt in range(NT):
                    for mt in (hp, KT + hp):
                        if hp == 0 and nt == 0:
                            continue  # prologue
                        # first S reading window nt of pair hp: b = 2nt
                        abunits.append(
                            (2 * nt * KT + hp,
                             lambda mt=mt, nt=nt: unit_a(mt, nt))
                        )
            abunits.sort(key=lambda u: u[0])
            dunits = []
            for nt in range(NT):
                for mt in range(KT):
                    # window nt's last norms (iter 12nt+11) are emitted
                    # during iteration 12nt+12, after its filler slot
                    dunits.append(
                        (12 * nt + 13, lambda nt=nt, mt=mt: unit_d(nt, mt))
                    )

            # ---- prologue ----
            for hp in range(KT):
                for jt in range(2):
                    (nc.scalar if hp % 2 == 0 else nc.gpsimd).dma_start(
                        out=bias_sb[(hp, jt)][:],
                        in_=bT[hp, jt * 128 : jt * 128 + JROWS[jt], :],
                    )
            for k in range(KT):
                nc.sync.dma_start(out=wpj_sb[k][:], in_=wpj[k])
            for jt in range(2):
                for n2 in range(2):
                    unit_b(0, jt, n2)
            unit_a(0, 0)
            unit_a(KT, 0)

            # ---- flat pipelined loop ----
            NITER = KT * BPC
            ai = di = 0
            for g in range(NITER + 1):
                # hard-due AB units must precede S[g]
                while ai < len(abunits) and abunits[ai][0] <= g:
                    abunits[ai][1]()
                    ai += 1
                if g < NITER:
                    emit_s(g)
                # soft top-up fillers between S[g] and AV[g-1]
                filled = 0
                while di < len(dunits) and dunits[di][0] <= g and filled < 2:
                    dunits[di][1]()
                    di += 1
                    filled += 1
                while filled < 2 and ai < len(abunits) and abunits[ai][0] <= g + 6:
                    abunits[ai][1]()
                    ai += 1
                    filled += 1
                if g >= 1:
                    emit_av(g - 1)
            while ai < len(abunits):
                abunits[ai][1]()
                ai += 1
            while di < len(dunits):
                dunits[di][1]()
                di += 1

    if finalize:
        nc.finalize()
    return nc


def _split8(a):
    """Error-compensated fp8 pair: a ~= hi + lo, each e4m3."""
    hi = a.astype(F8)
    lo = (a - hi.astype(np.float32)).astype(F8)
    return hi, lo


def _ktiles(a, nf):
    """(768, nf) -> (128, KT, nf) partition-major k-tiles."""
    return np.ascontiguousarray(a.reshape(KT, 128, nf).transpose(1, 0, 2))


def _host_prep(x, qkv_w, qkv_b, proj_w, proj_b, rel_table, log_temp, rel_index):
    """Build the per-core input maps (host-side layout prep only)."""
    x = np.asarray(x, np.float32)
    qkv_w = np.asarray(qkv_w, np.float32)
    qkv_b = np.asarray(qkv_b, np.float32)
    proj_w = np.asarray(proj_w, np.float32)
    rel_table = np.asarray(rel_table, np.float32)
    log_temp = np.asarray(log_temp, np.float32)
    rel_index = np.asarray(rel_index)

    temp = np.log1p(np.exp(log_temp.astype(np.float64))).astype(np.float32)
    alpha = (SCALE / temp).astype(np.float32)         # (H,) folded into q
    alpha_c = np.repeat(alpha, D)                     # (768,)

    # qk weights, host-scaled for fp8 range (SQ incl. alpha; SK plain),
    # split into hi/lo e4m3 pairs; hi-only and interleaved-cross layouts
    wqkT = qkv_w[0 : 2 * C].T.copy()                  # (768, 1536)
    wqkT[:, 0:C] *= alpha_c[None, :] * SQ
    wqkT[:, C : 2 * C] *= SK
    qhi, qlo = _split8(wqkT)
    qhi_t = _ktiles(qhi.astype(np.float32), 2 * C)
    qlo_t = _ktiles(qlo.astype(np.float32), 2 * C)
    wqk_hi_np = np.ascontiguousarray(
        qhi_t.reshape(128, KT // 2, 2, 2 * KT, 128).transpose(0, 3, 1, 2, 4)
    ).astype(F8)
    # cross weights LO-FIRST so the shared x cross buffer can stay HI-FIRST
    wqk_x_np = np.ascontiguousarray(
        np.stack([qlo_t, qhi_t], axis=2)
        .reshape(128, KT, 2, 2 * KT, 128)
        .transpose(0, 3, 1, 2, 4)
    ).astype(F8)

    # wv as fp8 hi/lo cross pairs, LO-FIRST (moving operand of V phase),
    # host-scaled by SVF out of the e4m3 subnormal range
    wvT = qkv_w[2 * C : 3 * C].T * SVF                # (768, 768)
    vhi, vlo = _split8(wvT)
    vhi_t = _ktiles(vhi.astype(np.float32), C)
    vlo_t = _ktiles(vlo.astype(np.float32), C)
    wvx = np.stack([vlo_t, vhi_t], axis=2)            # (128, KT, 2, C) lo-first
    wv_x_np = np.stack(
        [wvx[:, :, :, 0 : C // 2], wvx[:, :, :, C // 2 : C]], axis=0
    ).astype(F8)
    wpj_np = proj_w.T.reshape(KT, 128, C).astype(BF16)

    bq = qkv_b[0:C] * alpha_c
    bk = qkv_b[C : 2 * C]
    bqk_np = np.concatenate([bq, bk]).reshape(2 * KT, 128).T.copy().astype(np.float32)

    # multiplicative bias table: exp((relpos bias)/temp), diag -> 0, CLS -> 1,
    # transposed to (j, i); paired layout (KT, j, 2N)
    rpb = rel_table[rel_index]                        # (196, 196, H)
    bias = np.zeros((H, N, N), np.float32)
    bias[:, 1:, 1:] = rpb.transpose(2, 0, 1) / temp[:, None, None]
    ebias = np.exp(bias)
    idx = np.arange(1, N)
    ebias[:, idx, idx] = 0.0
    ebT = ebias.transpose(0, 2, 1)                    # (H, j, i)
    bT_np = (
        ebT.reshape(KT, 2, N, N).transpose(0, 2, 1, 3).reshape(KT, N, 2 * N)
    ).astype(BF16).copy()

    # window blocking: memory order [w0, w2, w1, w3], zero-padded to 128
    WOFF = (0, 197, 128, 325)
    WLEN = (128, 128, 69, 69)

    def _blocked(t, nunit):
        out = np.zeros((NT, 128, 4, nunit, 2, 128), np.float32)
        for nt in range(NT):
            for w in range(4):
                o = nt * TN + WOFF[w]
                out[nt, :, w, :, :, 0 : WLEN[w]] = t[:, :, :, o : o + WLEN[w]]
        return out.astype(F8)

    in_maps = []
    for c in range(NCORES):
        xc = x[c * BPC : (c + 1) * BPC].reshape(T, C).T  # (768, T)
        xhi, xlo = _split8(xc)
        xhi_t = _ktiles(xhi.astype(np.float32), T)      # (128, KT, T)
        xlo_t = _ktiles(xlo.astype(np.float32), T)
        xhi_p = xhi_t.reshape(128, KT // 2, 2, T)
        xx_p = np.stack([xhi_t, xlo_t], axis=2)         # (128, KT, 2, T)
        x_hi_np = _blocked(xhi_p, KT // 2)
        x_x_np = _blocked(xx_p, KT)
        in_maps.append(
            {
                "x_hi": x_hi_np,
                "x_x": x_x_np,
                "wv_x": wv_x_np,
                "wqk_hi": wqk_hi_np,
                "wqk_x": wqk_x_np,
                "wpj": wpj_np,
                "bT": bT_np,
                "bqk": bqk_np,
            }
        )
    return in_maps


def kernel(**inputs) -> np.ndarray:
    global LAST_RESULTS
    from concourse.bass_utils import run_bass_kernel_spmd

    if "nc" not in _CACHE:
        _CACHE["nc"] = _build()
    nc = _CACHE["nc"]

    in_maps = _host_prep(**inputs)
    try:
        res = run_bass_kernel_spmd(
            nc, in_maps, core_ids=list(range(NCORES)), trace=TRACE
        )
    except ModuleNotFoundError:
        res = run_bass_kernel_spmd(
            nc, in_maps, core_ids=list(range(NCORES)), trace=False
        )
    LAST_RESULTS = res

    # v-bias rides through attention unchanged (rows of attn sum to 1), so
    # its proj image folds into the constant output bias added here
    proj_b = np.asarray(inputs["proj_b"], np.float32)
    proj_w = np.asarray(inputs["proj_w"], np.float32)
    bv = np.asarray(inputs["qkv_b"], np.float32)[2 * C : 3 * C]
    b_eff = proj_b + proj_w @ bv
    outs = []
    for c in range(NCORES):
        oT = np.asarray(res.results[c]["outT"], np.float32).reshape(C, T)
        outs.append(oT.T.reshape(BPC, N, C))
    out = np.concatenate(outs, axis=0) + b_eff[None, None, :]
    return out.astype(np.float32)
